# revision 1
# baseline (speedup 1.0000x reference)
"""CASVDDenseMul fused kernel for 8 Trainium2 NeuronCores.

Reference computation (fp32):
    chi = sigmoid(context @ W + B)          # [B, R]
    t   = (inputs @ U) * (S * chi)          # [B, R]
    out = relu(t @ V.T + 2*bias)            # [B, UNITS]

Sharding: data-parallel over batch; each of the 8 cores handles 512 rows.
All factor weights (U, S, V, W, B, bias) are replicated.

Layout choices (all host-side, free):
  - x and context are fed transposed ([feature, batch]) so the contraction
    dim lands on SBUF partitions with no on-device transposes.
  - V is fed transposed ([R, UNITS]) so it can act as the matmul stationary
    operand per m-tile.
  - The kernel computes out.T [UNITS, batch] per core (bias is then a
    per-partition scalar, fusing bias+relu into one scalar-engine op);
    the host transposes back.
  - All matmul operands are pre-rounded to the fp32r grid (11-bit
    mantissa) on the host; the PE consumes fp32r at twice the fp32 rate.
  - U and xT are packed into one DRAM tensor ("ux"), W and ctxT into
    another ("wctx"), so each contraction chunk arrives with a single
    DMA (the Sync engine's ~0.6us per-DMA issue cost otherwise paces
    the whole input stream below HBM rate).
"""

import numpy as np

from concourse import bacc, mybir
from concourse import tile
from concourse.bass_utils import run_bass_kernel_spmd

N_CORES = 8
B_SZ, N_IN, N_CTX, UNITS, RANK = 4096, 4096, 512, 4096, 256
BS = B_SZ // N_CORES  # 512 batch rows per core

P = 128
KC_IN = N_IN // P    # 32 contraction chunks for x @ U
KC_CTX = N_CTX // P  # 4  contraction chunks for ctx @ W
RT = RANK // P       # 2  rank tiles
MT = UNITS // P      # 32 unit (output) tiles
UXW = RANK + BS      # 768 packed columns per k-chunk
# k-chunk DMA groups: big groups early (cheap issue), single chunks at the
# tail so the PE's last mm1 steps aren't gated on a wide transfer.
UX_GROUPS = [[k, k + 1] for k in range(0, 28, 2)] + [[28], [29], [30], [31]]

FP32 = mybir.dt.float32
FP32R = mybir.dt.float32r


def _build_nc():
    nc = bacc.Bacc("TRN2", target_bir_lowering=False, debug=False, enable_asserts=False)

    ux = nc.declare_dram_parameter("ux", [KC_IN, P, UXW], FP32R, isOutput=False)
    wctx = nc.declare_dram_parameter("wctx", [P, KC_CTX, UXW], FP32R, isOutput=False)
    VT = nc.declare_dram_parameter("VT", [RANK, UNITS], FP32R, isOutput=False)
    consts = nc.declare_dram_parameter("consts", [P, 2 * RT + MT], FP32, isOutput=False)
    outT = nc.declare_dram_parameter("outT", [UNITS, BS], FP32, isOutput=True)

    out_tiles = outT.rearrange("(m p) b -> m p b", p=P)   # [32, 128, 512]

    with tile.TileContext(nc) as tc:
        with (
            tc.tile_pool(name="cpool", bufs=1) as cpool,
            tc.tile_pool(name="small", bufs=1) as small,
            tc.tile_pool(name="stream", bufs=1) as stream,
            tc.tile_pool(name="acts", bufs=1) as acts,
            tc.tile_pool(name="ostage", bufs=8) as ostage,
            tc.tile_pool(name="pchi", bufs=2, space="PSUM") as pchi,
            tc.tile_pool(name="pt", bufs=1, space="PSUM") as pt,
            tc.tile_pool(name="pout", bufs=4, space="PSUM") as pout,
        ):
            # ---- small weights + constants ----
            wctx_sb = small.tile([P, KC_CTX, UXW], FP32R, tag="wctx")
            for k in range(KC_CTX):
                nc.sync.dma_start(wctx_sb[:, k, :], wctx[:, k, :])
            c_sb = cpool.tile([P, 2 * RT + MT], FP32, tag="consts")
            nc.sync.dma_start(c_sb[:], consts[:])
            s2_sb = c_sb[:, 0:RT]
            b2_sb = c_sb[:, RT:2 * RT]
            bias_sb = c_sb[:, 2 * RT:]

            # ---- PE warm-up: the HAM clock gate keeps the PE at 1.2 GHz
            # until it has been busy ~3.4us. Junk matmuls on a memset tile
            # during the (otherwise idle) DMA prologue flip it to 2.4 GHz
            # before the real stream begins.
            junk = acts.tile([P, BS], FP32, tag="junk")
            nc.gpsimd.memset(junk[:], 0.0)
            warm_ps = pchi.tile([P, BS], FP32, tag="chi", name="warm_ps")
            for _ in range(3):
                nc.tensor.matmul(
                    warm_ps[:],
                    junk[:, :P],
                    junk[:],
                    start=True,
                    stop=True,
                    skip_group_check=True,
                )

            # chi.T = sigmoid(W.T @ ctxT + B) * S -- emitted interleaved
            # with the mm1 stream below so its matmuls fill the PE's
            # DMA-wait gaps instead of delaying mm1's start.
            s_chi = [acts.tile([P, BS], FP32, tag=f"schi{rt}", name=f"schi{rt}")
                     for rt in range(RT)]

            def emit_chi(rt):
                psum_chi = pchi.tile([P, BS], FP32, tag="chi", name="psum_chi")
                for k in range(KC_CTX):
                    nc.tensor.matmul(
                        psum_chi[:],
                        wctx_sb[:, k, rt * P:(rt + 1) * P],
                        wctx_sb[:, k, RANK:],
                        start=(k == 0),
                        stop=(k == KC_CTX - 1),
                        skip_group_check=True,
                    )
                nc.scalar.activation(
                    s_chi[rt][:], psum_chi[:],
                    mybir.ActivationFunctionType.Sigmoid,
                    bias=b2_sb[:, rt:rt + 1], scale=1.0,
                )
                nc.vector.tensor_scalar_mul(
                    s_chi[rt][:], s_chi[rt][:], s2_sb[:, rt:rt + 1]
                )

            # ---- t.T = (U.T @ xT) * s_chi   (stream packed u/x groups) ----
            # VT rides the scalar HWDGE ring from the start (it is only
            # needed ~45us in, and keeping it out of the stream tail
            # stops the final x chunks crawling when the paired core's
            # stream overlaps); the u/x groups alternate rings behind it.
            ux_tiles = [stream.tile([P, len(ks), UXW], FP32R,
                                    tag=f"ux{g}", name=f"ux{g}")
                        for g, ks in enumerate(UX_GROUPS)]
            vt_sb = small.tile([P, RT, UNITS], FP32R, tag="vt")
            for c in range(RT):
                nc.scalar.dma_start(
                    vt_sb[:, c, :],
                    VT.rearrange("(c p) m -> c p m", p=P)[c],
                )
            # chi's sigmoids sit on the Scalar queue BEFORE the ux DMA
            # issues: the Scalar HWDGE ring is busy streaming VT then, so
            # the wait is free, and it keeps the chi->t' chain off the
            # critical path (otherwise sigmoid lands after ~50us of DMA
            # issues and delays mm2's start by ~3us).
            emit_chi(0)
            emit_chi(1)

            for g, ks in enumerate(UX_GROUPS):
                eng = nc.sync if g % 2 == 0 else nc.scalar
                eng.dma_start(
                    ux_tiles[g][:],
                    ux[ks[0]:ks[0] + len(ks)].rearrange("k p w -> p k w"),
                )

            psum_t = [pt.tile([P, BS], FP32, tag=f"t{rt}", name=f"pt{rt}")
                      for rt in range(RT)]
            HB = BS // 2
            for g, ks in enumerate(UX_GROUPS):
                for j, k in enumerate(ks):
                    if k == KC_IN - 1:
                        # final accumulation step in column halves, h0 first:
                        # t'-h0 (and the fast-start out tile) can begin while
                        # the h1 closing matmuls still stream.
                        for h in range(2):
                            for rt in range(RT):
                                nc.tensor.matmul(
                                    psum_t[rt][:, h * HB:(h + 1) * HB],
                                    ux_tiles[g][:, j, rt * P:(rt + 1) * P],
                                    ux_tiles[g][:, j, RANK + h * HB:RANK + (h + 1) * HB],
                                    start=False,
                                    stop=True,
                                    skip_group_check=True,
                                )
                    else:
                        for rt in range(RT):
                            nc.tensor.matmul(
                                psum_t[rt][:],
                                ux_tiles[g][:, j, rt * P:(rt + 1) * P],
                                ux_tiles[g][:, j, RANK:],
                                start=(k == 0),
                                stop=False,
                                skip_group_check=True,
                            )

            # t' in column halves: both rank-tiles' first halves are ready
            # after two 345ns DVE ops, so mm2's first tile can start ~0.7us
            # earlier than with full-width multiplies.
            H = BS // 2
            t_sb = [acts.tile([P, BS], FP32R, tag=f"t_sb{rt}", name=f"t_sb{rt}")
                    for rt in range(RT)]
            for h in range(2):
                for rt in range(RT):
                    sl = slice(h * H, (h + 1) * H)
                    nc.vector.tensor_mul(
                        t_sb[rt][:, sl], psum_t[rt][:, sl], s_chi[rt][:, sl]
                    )

            # ---- out.T = relu(V @ t.T + 2*bias) per 128-unit tile ----
            # fast-start: the first unit-tile runs at half batch width so its
            # matmuls/evac/DMA begin as soon as the first t' halves land
            for h in range(2):
                sl = slice(h * H, (h + 1) * H)
                ps0 = pout.tile([P, H], FP32, tag="o", name=f"po0h{h}")
                for c in range(RT):
                    nc.tensor.matmul(
                        ps0[:],
                        vt_sb[:, c, 0:P],
                        t_sb[c][:, sl],
                        start=(c == 0),
                        stop=(c == RT - 1),
                        skip_group_check=True,
                    )
                o0 = ostage.tile([P, H], FP32, tag="osb0", name=f"ot0h{h}")
                if h == 0:
                    nc.scalar.activation(
                        o0[:], ps0[:],
                        mybir.ActivationFunctionType.Relu,
                        bias=bias_sb[:, 0:1], scale=1.0,
                    )
                else:
                    nc.vector.tensor_scalar(
                        o0[:], ps0[:],
                        bias_sb[:, 0:1], 0.0,
                        op0=mybir.AluOpType.add, op1=mybir.AluOpType.max,
                    )
                eng = nc.sync if h == 0 else nc.scalar
                eng.dma_start(out_tiles[0][:, sl], o0[:])

            for m in range(1, MT):
                psum_o = pout.tile([P, BS], FP32, tag="o")
                for c in range(RT):
                    nc.tensor.matmul(
                        psum_o[:],
                        vt_sb[:, c, m * P:(m + 1) * P],
                        t_sb[c][:],
                        start=(c == 0),
                        stop=(c == RT - 1),
                    )
                o_sb = ostage.tile([P, BS], FP32, tag="osb")
                if m % 2 == 0:
                    nc.scalar.activation(
                        o_sb[:], psum_o[:],
                        mybir.ActivationFunctionType.Relu,
                        bias=bias_sb[:, m:m + 1], scale=1.0,
                    )
                else:
                    # split PSUM-evacuation load between ACT and DVE
                    nc.vector.tensor_scalar(
                        o_sb[:], psum_o[:],
                        bias_sb[:, m:m + 1], 0.0,
                        op0=mybir.AluOpType.add, op1=mybir.AluOpType.max,
                    )
                # alternate the two HWDGE rings so out-DMA issue keeps up
                eng = nc.sync if m % 2 == 0 else nc.scalar
                eng.dma_start(out_tiles[m], o_sb[:])

    nc.finalize()
    return nc


_NC_CACHE = {}


def _get_nc():
    if "nc" not in _NC_CACHE:
        _NC_CACHE["nc"] = _build_nc()
    return _NC_CACHE["nc"]


def _round_fp32r(a):
    """Round fp32 to the fp32r grid (11-bit mantissa; low 12 bits zero).

    The PE reads only the top 20 bits of each fp32r word; pre-rounding on
    the host gives round-to-nearest instead of hardware truncation.
    """
    u = np.ascontiguousarray(a, dtype=np.float32).view(np.uint32)
    r = (u + np.uint32(0x7FF) + ((u >> np.uint32(12)) & np.uint32(1))) & np.uint32(0xFFFFF000)
    return r.view(np.float32)


def _prepare_in_maps(inputs, context, U, S, V, W, B, bias):
    inputs = np.asarray(inputs, dtype=np.float32)
    context = np.asarray(context, dtype=np.float32)

    xTr = _round_fp32r(inputs.T)                 # [N_IN, B]
    ctxTr = _round_fp32r(context.T)              # [N_CTX, B]
    Ur = _round_fp32r(np.asarray(U, dtype=np.float32))
    Wr = _round_fp32r(np.asarray(W, dtype=np.float32))
    VTr = _round_fp32r(np.asarray(V, dtype=np.float32).T)  # [R, UNITS]

    S2 = np.asarray(S, dtype=np.float32).reshape(RT, P).T
    B2 = np.asarray(B, dtype=np.float32).reshape(RT, P).T
    bias2 = (2.0 * np.asarray(bias, dtype=np.float32)).reshape(MT, P).T
    consts = np.ascontiguousarray(
        np.concatenate([S2, B2, bias2], axis=1)
    )  # [128, 2+2+32]

    # U chunks, shared across cores: [KC_IN, P, RANK]
    u_g = Ur.reshape(KC_IN, P, RANK)
    # W chunks: [P, KC_CTX, RANK]
    w_g = Wr.reshape(KC_CTX, P, RANK).transpose(1, 0, 2)

    in_maps = []
    for c in range(N_CORES):
        sl = slice(c * BS, (c + 1) * BS)
        x_g = xTr[:, sl].reshape(KC_IN, P, BS)
        ux = np.empty((KC_IN, P, UXW), dtype=np.float32)
        ux[:, :, :RANK] = u_g
        ux[:, :, RANK:] = x_g
        ctx_g = ctxTr[:, sl].reshape(KC_CTX, P, BS).transpose(1, 0, 2)
        wctx = np.empty((P, KC_CTX, UXW), dtype=np.float32)
        wctx[:, :, :RANK] = w_g
        wctx[:, :, RANK:] = ctx_g
        in_maps.append({
            "ux": ux,
            "wctx": wctx,
            "VT": VTr,
            "consts": consts,
        })
    return in_maps


def _gather_out(results):
    out = np.empty((B_SZ, UNITS), dtype=np.float32)
    for c in range(N_CORES):
        out[c * BS:(c + 1) * BS, :] = results[c]["outT"].T
    return out


def kernel(inputs, context, U, S, V, W, B, bias):
    in_maps = _prepare_in_maps(inputs, context, U, S, V, W, B, bias)
    nc = _get_nc()
    res = run_bass_kernel_spmd(nc, in_maps, list(range(N_CORES)))
    return _gather_out(res.results)



# revision 4
# speedup vs baseline: 1.4307x; 1.4307x over previous
"""CASVDDenseMul fused kernel for 8 Trainium2 NeuronCores.

Reference computation (fp32):
    chi = sigmoid(context @ W + B)          # [B, R]
    t   = (inputs @ U) * (S * chi)          # [B, R]
    out = relu(t @ V.T + 2*bias)            # [B, UNITS]

Sharding: data-parallel over batch; each of the 8 cores handles 512 rows.
All factor weights (U, S, V, W, B, bias) are replicated.

This revision is bandwidth-first: the problem is DMA-bound (the PE array
runs one element/cell/cycle regardless of dtype, so fp32r and bf16 matmul
at the same rate -- but bf16 halves every byte moved).

  - All matmul operands and the output travel as bf16 (rel-err ~2e-3,
    well inside the 2e-2 gate). S is folded into U's columns on the host
    so the (S*chi) scale costs nothing on-device.
  - mm1 keeps U chunks stationary ([128,128] per rank-half) with x as the
    512/256-wide moving operand; the batch is split into two 256-column
    sub-blocks so the V-matmul for sub-block A runs in the PE's DMA-wait
    gaps while sub-block B is still streaming.
  - mm2 is flipped: t' ([rank,128-batch] tiles) is the stationary operand
    and VT streams through as the moving operand, so the output comes out
    in natural [batch, units] orientation (no host transpose) and VT is
    consumed piece-by-piece as it lands. The '+2*bias' is folded in as a
    1-partition accumulation matmul (ones-row x bias-row), emitted only
    when bias is nonzero.
  - Every DMA moves >=2KB per partition line (4KB for the streams), so
    descriptor-generation rate never caps throughput; the two HWDGE rings
    (sync + scalar) carry byte-balanced, consumption-ordered queues.
"""

import numpy as np
import ml_dtypes

from concourse import bacc, mybir
from concourse import tile
from concourse.bass_utils import run_bass_kernel_spmd

N_CORES = 8
B_SZ, N_IN, N_CTX, UNITS, RANK = 4096, 4096, 512, 4096, 256
BS = B_SZ // N_CORES   # 512 batch rows per core

P = 128
KC_IN = N_IN // P      # 32 contraction chunks for x @ U
KC_CTX = N_CTX // P    # 4  contraction chunks for ctx @ W
RT = RANK // P         # 2  rank tiles
NQ = 4                 # U/x stream pieces (8 chunks each)
KPQ = KC_IN // NQ      # 8 chunks per piece
NSB = 2                # batch sub-blocks
BSB = BS // NSB        # 256 batch cols per sub-block
NBT = BSB // P         # 2 batch tiles (128) per sub-block
NW = 4                 # VT pieces / unit waves (1024 units each)
WU = UNITS // NW       # 1024 units per wave

BF16 = mybir.dt.bfloat16
FP32 = mybir.dt.float32
FP32R = mybir.dt.float32r

bf16 = ml_dtypes.bfloat16


def _build_nc(use_b, use_bias):
    nc = bacc.Bacc("TRN2", target_bir_lowering=False, debug=False, enable_asserts=False)

    wctx = nc.declare_dram_parameter("wctx", [P, KC_CTX * (RANK + BS)], BF16, isOutput=False)
    u4 = nc.declare_dram_parameter("u4", [NQ, P, KPQ * RANK], BF16, isOutput=False)
    xg = nc.declare_dram_parameter("xg", [NSB, NQ, P, KPQ * BSB], BF16, isOutput=False)
    vt4 = nc.declare_dram_parameter("vt4", [NW, P, RT * WU], BF16, isOutput=False)
    if use_b:
        bvec = nc.declare_dram_parameter("bvec", [P, RT], FP32, isOutput=False)
    if use_bias:
        brow = nc.declare_dram_parameter("brow", [1, P + UNITS], FP32R, isOutput=False)
    out_d = nc.declare_dram_parameter("out_d", [BS, UNITS], BF16, isOutput=True)

    with tile.TileContext(nc) as tc:
        with (
            tc.tile_pool(name="small", bufs=1) as small,
            tc.tile_pool(name="stream", bufs=1) as stream,
            tc.tile_pool(name="acts", bufs=1) as acts,
            tc.tile_pool(name="ostage", bufs=4) as ostage,
            tc.tile_pool(name="pchi", bufs=1, space="PSUM") as pchi,
            tc.tile_pool(name="pt", bufs=1, space="PSUM") as pt,
            tc.tile_pool(name="pout", bufs=2, space="PSUM") as pout,
        ):
            # ---- SBUF tiles ----
            wctx_sb = small.tile([P, KC_CTX * (RANK + BS)], BF16, tag="wctx")
            u_sb = small.tile([P, NQ, KPQ * RANK], BF16, tag="u")
            x_sb = [[stream.tile([P, KPQ * BSB], BF16, tag=f"x{s}{g}", name=f"x{s}{g}")
                     for g in range(NQ)] for s in range(NSB)]
            vt_sb = small.tile([P, NW, RT * WU], BF16, tag="vt")
            if use_b:
                bvec_sb = small.tile([P, RT], FP32, tag="bvec")
            if use_bias:
                brow_sb = small.tile([1, P + UNITS], FP32R, tag="brow")
            s_chi = acts.tile([P, RT, BS], FP32, tag="schi")
            t_sb = [acts.tile([P, RT, BSB], BF16, tag=f"tsb{s}", name=f"tsb{s}")
                    for s in range(NSB)]
            junk = acts.tile([P, BS], BF16, tag="junk")

            # ---- DMA issue queues (order per ring == consumption order) ----
            # ring S (sync):   wctx, bvec, u0, xA1, u2, xA3, vt1, vt3, xB1, xB3
            # ring C (scalar): brow, xA0, u1, xA2, u3, vt0, vt2, xB0, xB2
            nc.sync.dma_start(wctx_sb[:], wctx[:])
            if use_b:
                nc.sync.dma_start(bvec_sb[:], bvec[:])
            if use_bias:
                nc.scalar.dma_start(brow_sb[:], brow[:])
            nc.scalar.dma_start(x_sb[0][0][:], xg[0, 0])
            nc.sync.dma_start(u_sb[:, 0, :], u4[0])
            nc.scalar.dma_start(u_sb[:, 1, :], u4[1])
            nc.sync.dma_start(x_sb[0][1][:], xg[0, 1])
            nc.scalar.dma_start(x_sb[0][2][:], xg[0, 2])
            nc.sync.dma_start(u_sb[:, 2, :], u4[2])
            nc.scalar.dma_start(u_sb[:, 3, :], u4[3])
            nc.sync.dma_start(x_sb[0][3][:], xg[0, 3])
            nc.scalar.dma_start(vt_sb[:, 0, :], vt4[0])
            nc.sync.dma_start(vt_sb[:, 1, :], vt4[1])
            nc.scalar.dma_start(vt_sb[:, 2, :], vt4[2])
            nc.sync.dma_start(vt_sb[:, 3, :], vt4[3])
            nc.scalar.dma_start(x_sb[1][0][:], xg[1, 0])
            nc.sync.dma_start(x_sb[1][1][:], xg[1, 1])
            nc.scalar.dma_start(x_sb[1][2][:], xg[1, 2])
            nc.sync.dma_start(x_sb[1][3][:], xg[1, 3])

            # ---- PE warm-up: keep the HAM activity window busy from t=0
            # so the clock gate lifts to 2.4 GHz before the real stream.
            nc.gpsimd.memset(junk[:], 0.0)
            warm_ps = pchi.tile([P, BS], FP32, tag="chi", name="warm_ps")
            for _ in range(8):
                nc.tensor.matmul(
                    warm_ps[:], junk[:, :P], junk[:],
                    start=True, stop=True, skip_group_check=True,
                )

            # ---- chi' = sigmoid(W.T @ ctxT + B)  (S folded into U) ----
            for rt in range(RT):
                psum_chi = pchi.tile([P, BS], FP32, tag="chi", name=f"pchi{rt}")
                for k in range(KC_CTX):
                    base = k * (RANK + BS)
                    nc.tensor.matmul(
                        psum_chi[:],
                        wctx_sb[:, base + rt * P: base + (rt + 1) * P],
                        wctx_sb[:, base + RANK: base + RANK + BS],
                        start=(k == 0), stop=(k == KC_CTX - 1),
                        skip_group_check=True,
                    )
                nc.scalar.activation(
                    s_chi[:, rt, :], psum_chi[:],
                    mybir.ActivationFunctionType.Sigmoid,
                    bias=(bvec_sb[:, rt:rt + 1] if use_b else 0.0), scale=1.0,
                )

            psum_t = [pt.tile([P, RT * BSB], FP32, tag=f"pt{s}", name=f"pt{s}")
                      for s in range(NSB)]

            def emit_mm1(s):
                # psum_t[s][:, rt*BSB:(rt+1)*BSB] += U'_k.T @ x_k  over all k.
                # Both rank-half groups live in ONE psum bank; start=True
                # clears has_written BANK-wide, so only the very first matmul
                # may carry it -- the rt1 group's k=0 lands on cleared
                # has_written bits and start=False already overwrites.
                for k in range(KC_IN):
                    q, j = divmod(k, KPQ)
                    for rt in range(RT):
                        nc.tensor.matmul(
                            psum_t[s][:, rt * BSB:(rt + 1) * BSB],
                            u_sb[:, q, j * RANK + rt * P: j * RANK + (rt + 1) * P],
                            x_sb[s][q][:, j * BSB:(j + 1) * BSB],
                            start=(k == 0 and rt == 0),
                            stop=(k == KC_IN - 1),
                            skip_group_check=True,
                        )

            def emit_tprime(s):
                for rt in range(RT):
                    nc.vector.tensor_mul(
                        t_sb[s][:, rt, :],
                        psum_t[s][:, rt * BSB:(rt + 1) * BSB],
                        s_chi[:, rt, s * BSB:(s + 1) * BSB],
                    )

            def emit_mm2(s, wave_order):
                # out[bt-rows, wave-units] = t'.T @ VT (+ 2*bias), relu, DMA.
                for wi, w in enumerate(wave_order):
                    for bt in range(NBT):
                        pw = pout.tile([P, WU], FP32, tag="po")
                        for rt in range(RT):
                            for h in range(2):
                                nc.tensor.matmul(
                                    pw[:, h * 512:(h + 1) * 512],
                                    t_sb[s][:, rt, bt * P:(bt + 1) * P],
                                    vt_sb[:, w, rt * WU + h * 512: rt * WU + (h + 1) * 512],
                                    start=(rt == 0),
                                    stop=(rt == RT - 1 and not use_bias),
                                    skip_group_check=True,
                                )
                        if use_bias:
                            for h in range(2):
                                uc = w * 2 + h
                                nc.tensor.matmul(
                                    pw[:, h * 512:(h + 1) * 512],
                                    brow_sb[:, 0:P],
                                    brow_sb[:, P + uc * 512: P + (uc + 1) * 512],
                                    start=False, stop=True,
                                    skip_group_check=True,
                                )
                        o_sb = ostage.tile([P, WU], BF16, tag="osb")
                        widx = wi * NBT + bt
                        if widx % 2 == 0:
                            nc.scalar.activation(
                                o_sb[:], pw[:],
                                mybir.ActivationFunctionType.Relu,
                            )
                        else:
                            nc.vector.tensor_scalar(
                                o_sb[:], pw[:], 0.0, None,
                                op0=mybir.AluOpType.max,
                            )
                        rows = slice(s * BSB + bt * P, s * BSB + (bt + 1) * P)
                        cols = slice(w * WU, (w + 1) * WU)
                        eng = nc.scalar if (s == 0) != (widx % 2 == 0) else nc.sync
                        eng.dma_start(out_d[rows, cols], o_sb[:])

            emit_mm1(0)
            emit_tprime(0)
            # vt pieces land in ring order 0, 2, 1, 3 -> consume in that order
            emit_mm2(0, [0, 2, 1, 3])
            emit_mm1(1)
            emit_tprime(1)
            emit_mm2(1, [0, 1, 2, 3])

    nc.finalize()
    return nc


_NC_CACHE = {}


def _get_nc(use_b=False, use_bias=False):
    key = (use_b, use_bias)
    if key not in _NC_CACHE:
        _NC_CACHE[key] = _build_nc(use_b, use_bias)
    return _NC_CACHE[key]


def _round_fp32r(a):
    """Round fp32 to the fp32r grid (11-bit mantissa; low 12 bits zero)."""
    u = np.ascontiguousarray(a, dtype=np.float32).view(np.uint32)
    r = (u + np.uint32(0x7FF) + ((u >> np.uint32(12)) & np.uint32(1))) & np.uint32(0xFFFFF000)
    return r.view(np.float32)


def build(inputs, context, U, S, V, W, B, bias):
    """Host-side packing: returns (nc, in_maps)."""
    use_b = bool(np.any(np.asarray(B)))
    use_bias = bool(np.any(np.asarray(bias)))

    # U with S folded into its columns, chunked for the stream:
    # u4[q, p, j*RANK + r] = (U*S)[(q*KPQ+j)*128 + p, r]
    US = (np.asarray(U, np.float32) * np.asarray(S, np.float32)[None, :]).astype(bf16)
    u4 = np.ascontiguousarray(
        US.reshape(NQ, KPQ, P, RANK).transpose(0, 2, 1, 3).reshape(NQ, P, KPQ * RANK)
    )

    # VT pieces: vt4[c, p, rt*WU + m'] = V.T[rt*128 + p, c*WU + m']
    VTb = np.asarray(V, np.float32).T.astype(bf16)          # [RANK, UNITS]
    vt4 = np.ascontiguousarray(
        VTb.reshape(RT, P, NW, WU).transpose(2, 1, 0, 3).reshape(NW, P, RT * WU)
    )

    Wk = np.asarray(W, np.float32).astype(bf16).reshape(KC_CTX, P, RANK)
    ctxT = np.asarray(context, np.float32).astype(bf16).T   # [N_CTX, B_SZ]
    xT = np.asarray(inputs, np.float32).astype(bf16).T      # [N_IN, B_SZ]

    bvec = np.ascontiguousarray(np.asarray(B, np.float32).reshape(RT, P).T)
    brow = np.empty((1, P + UNITS), np.float32)
    brow[0, :P] = 1.0
    brow[0, P:] = 2.0 * np.asarray(bias, np.float32)
    brow = _round_fp32r(brow)

    in_maps = []
    for c in range(N_CORES):
        sl = slice(c * BS, (c + 1) * BS)
        # wctx[p, k*(RANK+BS) + ...] = [W_k | ctx_k] per contraction chunk
        wc = np.empty((KC_CTX, P, RANK + BS), bf16)
        wc[:, :, :RANK] = Wk
        wc[:, :, RANK:] = ctxT[:, sl].reshape(KC_CTX, P, BS)
        wctx = np.ascontiguousarray(
            wc.transpose(1, 0, 2).reshape(P, KC_CTX * (RANK + BS))
        )
        # xg[s, g, p, j*BSB + b'] = xT[(g*KPQ+j)*128 + p, c*BS + s*BSB + b']
        xc = xT[:, sl]                                       # [N_IN, BS]
        xgc = np.ascontiguousarray(
            xc.reshape(NQ, KPQ, P, NSB, BSB)
              .transpose(3, 0, 2, 1, 4)
              .reshape(NSB, NQ, P, KPQ * BSB)
        )
        m = {"wctx": wctx, "u4": u4, "xg": xgc, "vt4": vt4}
        if use_b:
            m["bvec"] = bvec
        if use_bias:
            m["brow"] = brow
        in_maps.append(m)
    return _get_nc(use_b, use_bias), in_maps


def gather_out(results):
    out = np.empty((B_SZ, UNITS), dtype=np.float32)
    for c in range(N_CORES):
        out[c * BS:(c + 1) * BS, :] = results[c]["out_d"].astype(np.float32)
    return out


def kernel(inputs, context, U, S, V, W, B, bias):
    nc, in_maps = build(inputs, context, U, S, V, W, B, bias)
    res = run_bass_kernel_spmd(nc, in_maps, list(range(N_CORES)))
    return gather_out(res.results)


# revision 6
# speedup vs baseline: 1.4845x; 1.0376x over previous
"""CASVDDenseMul fused kernel for 8 Trainium2 NeuronCores.

Reference computation (fp32):
    chi = sigmoid(context @ W + B)          # [B, R]
    t   = (inputs @ U) * (S * chi)          # [B, R]
    out = relu(t @ V.T + 2*bias)            # [B, UNITS]

Sharding: data-parallel over batch; each of the 8 cores handles 512 rows.
All factor weights (U, S, V, W, B, bias) are replicated.

Design notes (v3 -- PE-dense):
  - Everything travels as bf16 (the PE runs one element/cell/cycle for
    any dtype, so bf16 matmuls at fp32r speed while halving DMA bytes;
    rel-err ~3e-3 vs the 2e-2 gate). S is folded into U's columns on the
    host.
  - The per-core PE work (~29us of matmul streaming) exceeds the input
    stream time (~22us at ~410GB/s), so the kernel is shaped to keep the
    PE gapless: U/x stream in 0.26MB pieces so mm1 starts ~1.3us after
    the first descriptor lands; chi fills mm1's DMA-wait gaps; the
    sub-block-B mm1 is interleaved with sub-block-A's V-matmul waves; VT
    streams mid-stream and is consumed piece-by-piece as the moving
    operand of mm2 (t' is stationary), which also lands the output in
    natural [batch, units] orientation.
  - mm2 uses 1024-wide bf16 moving operands (one accumulation group per
    2-bank PSUM tile), relu-evacuated alternately by ACT and DVE, with
    per-wave 0.26MB output writes so the write tail after the last
    matmul is short.
  - PSUM note: start=True clears has_written BANK-wide, so only the
    first matmul into a shared bank carries start=True (mm1's second
    rank-half group relies on the cleared bits to overwrite on its first
    accumulation step).
"""

import numpy as np
import ml_dtypes

from concourse import bacc, mybir
from concourse import tile
from concourse.bass_utils import run_bass_kernel_spmd

N_CORES = 8
B_SZ, N_IN, N_CTX, UNITS, RANK = 4096, 4096, 512, 4096, 256
BS = B_SZ // N_CORES   # 512 batch rows per core

P = 128
KC_IN = N_IN // P      # 32 contraction chunks for x @ U
KC_CTX = N_CTX // P    # 4  contraction chunks for ctx @ W
RT = RANK // P         # 2  rank tiles
NQ = 8                 # U/x stream pieces
KPQ = KC_IN // NQ      # 4 chunks per piece
NSB = 2                # batch sub-blocks
BSB = BS // NSB        # 256 batch cols per sub-block
NBT = BSB // P         # 2 batch tiles (128) per sub-block
NW = 4                 # VT pieces / unit waves (1024 units each)
WU = UNITS // NW       # 1024 units per wave

BF16 = mybir.dt.bfloat16
FP32 = mybir.dt.float32
FP32R = mybir.dt.float32r

bf16 = ml_dtypes.bfloat16


def _build_nc(use_b, use_bias):
    nc = bacc.Bacc("TRN2", target_bir_lowering=False, debug=False, enable_asserts=False)

    wctx = nc.declare_dram_parameter("wctx", [P, KC_CTX * (RANK + BS)], BF16, isOutput=False)
    u8 = nc.declare_dram_parameter("u8", [NQ, P, KPQ * RANK], BF16, isOutput=False)
    xg = nc.declare_dram_parameter("xg", [NSB, NQ, P, KPQ * BSB], BF16, isOutput=False)
    vt4 = nc.declare_dram_parameter("vt4", [NW, P, RT * WU], BF16, isOutput=False)
    if use_b:
        bvec = nc.declare_dram_parameter("bvec", [P, RT], FP32, isOutput=False)
    if use_bias:
        brow = nc.declare_dram_parameter("brow", [1, P + UNITS], FP32R, isOutput=False)
    out_d = nc.declare_dram_parameter("out_d", [BS, UNITS], BF16, isOutput=True)

    with tile.TileContext(nc) as tc:
        with (
            tc.tile_pool(name="small", bufs=1) as small,
            tc.tile_pool(name="stream", bufs=1) as stream,
            tc.tile_pool(name="acts", bufs=1) as acts,
            tc.tile_pool(name="ostage", bufs=4) as ostage,
            tc.tile_pool(name="pchi", bufs=1, space="PSUM") as pchi,
            tc.tile_pool(name="pt", bufs=1, space="PSUM") as pt,
            tc.tile_pool(name="pout", bufs=2, space="PSUM") as pout,
        ):
            # ---- SBUF tiles ----
            wctx_sb = small.tile([P, KC_CTX * (RANK + BS)], BF16, tag="wctx")
            u_sb = small.tile([P, NQ, KPQ * RANK], BF16, tag="u")
            x_sb = [[stream.tile([P, KPQ * BSB], BF16, tag=f"x{s}{g}", name=f"x{s}{g}")
                     for g in range(NQ)] for s in range(NSB)]
            vt_sb = small.tile([P, NW, RT * WU], BF16, tag="vt")
            if use_b:
                bvec_sb = small.tile([P, RT], FP32, tag="bvec")
            if use_bias:
                brow_sb = small.tile([1, P + UNITS], FP32R, tag="brow")
            s_chi = acts.tile([P, RT, BS], FP32, tag="schi")
            t_sb = [acts.tile([P, RT, BSB], BF16, tag=f"tsb{s}", name=f"tsb{s}")
                    for s in range(NSB)]
            junk = acts.tile([P, P], BF16, tag="junk")

            # ---- DMA issue queues (per-ring order == consumption order).
            # u/xA piece-pairs alternate rings so mm1 piece k needs only the
            # k-th completion on each ring; wctx mid-stream (chi is gap
            # filler); VT next (mm2-A waves); xB last (mm1-B interleaves).
            nc.sync.dma_start(u_sb[:, 0, :], u8[0])
            nc.scalar.dma_start(x_sb[0][0][:], xg[0, 0])
            nc.sync.dma_start(x_sb[0][1][:], xg[0, 1])
            nc.scalar.dma_start(u_sb[:, 1, :], u8[1])
            nc.sync.dma_start(u_sb[:, 2, :], u8[2])
            nc.scalar.dma_start(x_sb[0][2][:], xg[0, 2])
            nc.sync.dma_start(x_sb[0][3][:], xg[0, 3])
            nc.scalar.dma_start(u_sb[:, 3, :], u8[3])
            nc.sync.dma_start(wctx_sb[:], wctx[:])
            if use_b:
                nc.scalar.dma_start(bvec_sb[:], bvec[:])
            if use_bias:
                nc.scalar.dma_start(brow_sb[:], brow[:])
            nc.scalar.dma_start(u_sb[:, 4, :], u8[4])
            nc.sync.dma_start(x_sb[0][4][:], xg[0, 4])
            nc.scalar.dma_start(x_sb[0][5][:], xg[0, 5])
            nc.sync.dma_start(u_sb[:, 5, :], u8[5])
            nc.scalar.dma_start(u_sb[:, 6, :], u8[6])
            nc.sync.dma_start(x_sb[0][6][:], xg[0, 6])
            nc.scalar.dma_start(x_sb[0][7][:], xg[0, 7])
            nc.sync.dma_start(u_sb[:, 7, :], u8[7])
            nc.scalar.dma_start(vt_sb[:, 0, :], vt4[0])
            nc.sync.dma_start(vt_sb[:, 1, :], vt4[1])
            nc.scalar.dma_start(vt_sb[:, 2, :], vt4[2])
            nc.sync.dma_start(vt_sb[:, 3, :], vt4[3])
            for g in range(NQ):
                eng = nc.scalar if g % 2 == 0 else nc.sync
                eng.dma_start(x_sb[1][g][:], xg[1, g])

            # ---- PE warm-up: keep the HAM activity window busy from t=0
            # so the clock gate lifts to 2.4 GHz before the real stream.
            nc.gpsimd.memset(junk[:], 0.0)
            warm_ps = pchi.tile([P, BS], FP32, tag="chi", name="warm_ps")
            for _ in range(6):
                nc.tensor.matmul(
                    warm_ps[:, :P], junk[:], junk[:],
                    start=True, stop=True, skip_group_check=True,
                )

            # ---- chi' = sigmoid(W.T @ ctxT + B)  (S folded into U) ----
            for rt in range(RT):
                psum_chi = pchi.tile([P, BS], FP32, tag="chi", name=f"pchi{rt}")
                for k in range(KC_CTX):
                    base = k * (RANK + BS)
                    nc.tensor.matmul(
                        psum_chi[:],
                        wctx_sb[:, base + rt * P: base + (rt + 1) * P],
                        wctx_sb[:, base + RANK: base + RANK + BS],
                        start=(k == 0), stop=(k == KC_CTX - 1),
                        skip_group_check=True,
                    )
                nc.scalar.activation(
                    s_chi[:, rt, :], psum_chi[:],
                    mybir.ActivationFunctionType.Sigmoid,
                    bias=(bvec_sb[:, rt:rt + 1] if use_b else 0.0), scale=1.0,
                )

            psum_t = [pt.tile([P, RT * BSB], FP32, tag=f"pt{s}", name=f"pt{s}")
                      for s in range(NSB)]

            def emit_mm1_piece(s, q):
                # psum_t[s][:, rt*BSB:(rt+1)*BSB] += U'_k.T @ x_k, k in piece q.
                # Both rank-half groups share ONE psum bank; start=True clears
                # has_written BANK-wide, so only the very first matmul carries
                # it (the rt1 group's k=0 lands on cleared bits and start=False
                # already overwrites).
                for j in range(KPQ):
                    k = q * KPQ + j
                    for rt in range(RT):
                        nc.tensor.matmul(
                            psum_t[s][:, rt * BSB:(rt + 1) * BSB],
                            u_sb[:, q, j * RANK + rt * P: j * RANK + (rt + 1) * P],
                            x_sb[s][q][:, j * BSB:(j + 1) * BSB],
                            start=(k == 0 and rt == 0),
                            stop=(k == KC_IN - 1),
                            skip_group_check=True,
                        )

            def emit_tprime(s):
                for rt in range(RT):
                    nc.vector.tensor_mul(
                        t_sb[s][:, rt, :],
                        psum_t[s][:, rt * BSB:(rt + 1) * BSB],
                        s_chi[:, rt, s * BSB:(s + 1) * BSB],
                    )

            def emit_mm2_wave(s, w, bt, widx):
                # out[bt-rows, wave-units] = t'.T @ VT (+ 2*bias), relu, DMA.
                # One accumulation group per 2-bank tile (N=1024 moving).
                pw = pout.tile([P, WU], FP32, tag="po")
                for rt in range(RT):
                    for h in range(2):
                        nc.tensor.matmul(
                            pw[:, h * 512:(h + 1) * 512],
                            t_sb[s][:, rt, bt * P:(bt + 1) * P],
                            vt_sb[:, w, rt * WU + h * 512: rt * WU + (h + 1) * 512],
                            start=(rt == 0),
                            stop=(rt == RT - 1 and not use_bias),
                            skip_group_check=True,
                        )
                if use_bias:
                    for h in range(2):
                        nc.tensor.matmul(
                            pw[:, h * 512:(h + 1) * 512],
                            brow_sb[:, 0:P],
                            brow_sb[:, P + w * WU + h * 512: P + w * WU + (h + 1) * 512],
                            start=False, stop=True,
                            skip_group_check=True,
                        )
                o_sb = ostage.tile([P, WU], BF16, tag="osb")
                if widx % 2 == 0:
                    nc.scalar.activation(
                        o_sb[:], pw[:], mybir.ActivationFunctionType.Relu,
                    )
                else:
                    nc.vector.tensor_scalar(
                        o_sb[:], pw[:], 0.0, None, op0=mybir.AluOpType.max,
                    )
                rows = slice(s * BSB + bt * P, s * BSB + (bt + 1) * P)
                cols = slice(w * WU, (w + 1) * WU)
                eng = nc.scalar if (s == 0) != (widx % 2 == 0) else nc.sync
                eng.dma_start(out_d[rows, cols], o_sb[:])

            # mm1-A (DMA-paced, chi fills its gaps)
            for q in range(NQ):
                emit_mm1_piece(0, q)
            emit_tprime(0)
            # mm2-A waves interleaved with mm1-B pieces: the PE alternates
            # between VT-fed waves and freshly-landed xB pieces, staying
            # gapless while the stream tail arrives.
            widx = 0
            for i, w in enumerate([0, 2, 1, 3]):
                for bt in range(NBT):
                    emit_mm2_wave(0, w, bt, widx)
                    widx += 1
                emit_mm1_piece(1, 2 * i)
                emit_mm1_piece(1, 2 * i + 1)
            emit_tprime(1)
            widx = 0
            for w in range(NW):
                for bt in range(NBT):
                    emit_mm2_wave(1, w, bt, widx)
                    widx += 1

    nc.finalize()
    return nc


_NC_CACHE = {}


def _get_nc(use_b=False, use_bias=False):
    key = (use_b, use_bias)
    if key not in _NC_CACHE:
        _NC_CACHE[key] = _build_nc(use_b, use_bias)
    return _NC_CACHE[key]


def _round_fp32r(a):
    """Round fp32 to the fp32r grid (11-bit mantissa; low 12 bits zero)."""
    u = np.ascontiguousarray(a, dtype=np.float32).view(np.uint32)
    r = (u + np.uint32(0x7FF) + ((u >> np.uint32(12)) & np.uint32(1))) & np.uint32(0xFFFFF000)
    return r.view(np.float32)


def build(inputs, context, U, S, V, W, B, bias):
    """Host-side packing: returns (nc, in_maps)."""
    use_b = bool(np.any(np.asarray(B)))
    use_bias = bool(np.any(np.asarray(bias)))

    # U with S folded into its columns, chunked for the stream:
    # u8[q, p, j*RANK + r] = (U*S)[(q*KPQ+j)*128 + p, r]
    US = (np.asarray(U, np.float32) * np.asarray(S, np.float32)[None, :]).astype(bf16)
    u8 = np.ascontiguousarray(
        US.reshape(NQ, KPQ, P, RANK).transpose(0, 2, 1, 3).reshape(NQ, P, KPQ * RANK)
    )

    # VT pieces: vt4[c, p, rt*WU + m'] = V.T[rt*128 + p, c*WU + m']
    VTb = np.asarray(V, np.float32).T.astype(bf16)          # [RANK, UNITS]
    vt4 = np.ascontiguousarray(
        VTb.reshape(RT, P, NW, WU).transpose(2, 1, 0, 3).reshape(NW, P, RT * WU)
    )

    Wk = np.asarray(W, np.float32).astype(bf16).reshape(KC_CTX, P, RANK)
    ctxT = np.asarray(context, np.float32).astype(bf16).T   # [N_CTX, B_SZ]
    xT = np.asarray(inputs, np.float32).astype(bf16).T      # [N_IN, B_SZ]

    bvec = np.ascontiguousarray(np.asarray(B, np.float32).reshape(RT, P).T)
    brow = np.empty((1, P + UNITS), np.float32)
    brow[0, :P] = 1.0
    brow[0, P:] = 2.0 * np.asarray(bias, np.float32)
    brow = _round_fp32r(brow)

    in_maps = []
    for c in range(N_CORES):
        sl = slice(c * BS, (c + 1) * BS)
        # wctx[p, k*(RANK+BS) + ...] = [W_k | ctx_k] per contraction chunk
        wc = np.empty((KC_CTX, P, RANK + BS), bf16)
        wc[:, :, :RANK] = Wk
        wc[:, :, RANK:] = ctxT[:, sl].reshape(KC_CTX, P, BS)
        wctx = np.ascontiguousarray(
            wc.transpose(1, 0, 2).reshape(P, KC_CTX * (RANK + BS))
        )
        # xg[s, g, p, j*BSB + b'] = xT[(g*KPQ+j)*128 + p, c*BS + s*BSB + b']
        xc = xT[:, sl]                                       # [N_IN, BS]
        xgc = np.ascontiguousarray(
            xc.reshape(NQ, KPQ, P, NSB, BSB)
              .transpose(3, 0, 2, 1, 4)
              .reshape(NSB, NQ, P, KPQ * BSB)
        )
        m = {"wctx": wctx, "u8": u8, "xg": xgc, "vt4": vt4}
        if use_b:
            m["bvec"] = bvec
        if use_bias:
            m["brow"] = brow
        in_maps.append(m)
    return _get_nc(use_b, use_bias), in_maps


def gather_out(results):
    out = np.empty((B_SZ, UNITS), dtype=np.float32)
    for c in range(N_CORES):
        out[c * BS:(c + 1) * BS, :] = results[c]["out_d"].astype(np.float32)
    return out


def kernel(inputs, context, U, S, V, W, B, bias):
    nc, in_maps = build(inputs, context, U, S, V, W, B, bias)
    res = run_bass_kernel_spmd(nc, in_maps, list(range(N_CORES)))
    return gather_out(res.results)


# revision 9
# speedup vs baseline: 1.5676x; 1.0560x over previous
"""CASVDDenseMul fused kernel for 8 Trainium2 NeuronCores.

Reference computation (fp32):
    chi = sigmoid(context @ W + B)          # [B, R]
    t   = (inputs @ U) * (S * chi)          # [B, R]
    out = relu(t @ V.T + 2*bias)            # [B, UNITS]

Sharding: data-parallel over batch; each of the 8 cores handles 512 rows.
All factor weights (U, S, V, W, B, bias) are replicated.

Design notes (v3 -- PE-dense):
  - Everything travels as bf16 (the PE runs one element/cell/cycle for
    any dtype, so bf16 matmuls at fp32r speed while halving DMA bytes;
    rel-err ~3e-3 vs the 2e-2 gate). S is folded into U's columns on the
    host.
  - The per-core PE work (~29us of matmul streaming) exceeds the input
    stream time (~22us at ~410GB/s), so the kernel is shaped to keep the
    PE gapless: U/x stream in 0.26MB pieces so mm1 starts ~1.3us after
    the first descriptor lands; chi fills mm1's DMA-wait gaps; the
    sub-block-B mm1 is interleaved with sub-block-A's V-matmul waves; VT
    streams mid-stream and is consumed piece-by-piece as the moving
    operand of mm2 (t' is stationary), which also lands the output in
    natural [batch, units] orientation.
  - mm2 uses 1024-wide bf16 moving operands (one accumulation group per
    2-bank PSUM tile), relu-evacuated alternately by ACT and DVE, with
    per-wave 0.26MB output writes so the write tail after the last
    matmul is short.
  - PSUM note: start=True clears has_written BANK-wide, so only the
    first matmul into a shared bank carries start=True (mm1's second
    rank-half group relies on the cleared bits to overwrite on its first
    accumulation step).
"""

import numpy as np
import ml_dtypes

from concourse import bacc, mybir
from concourse import tile
from concourse.bass_utils import run_bass_kernel_spmd

N_CORES = 8
B_SZ, N_IN, N_CTX, UNITS, RANK = 4096, 4096, 512, 4096, 256
BS = B_SZ // N_CORES   # 512 batch rows per core

P = 128
KC_IN = N_IN // P      # 32 contraction chunks for x @ U
KC_CTX = N_CTX // P    # 4  contraction chunks for ctx @ W
RT = RANK // P         # 2  rank tiles
NQ = 8                 # U/x stream pieces
KPQ = KC_IN // NQ      # 4 chunks per piece
NSB = 2                # batch sub-blocks
BSB = BS // NSB        # 256 batch cols per sub-block
NBT = BSB // P         # 2 batch tiles (128) per sub-block
NW = 4                 # VT pieces / unit waves (1024 units each)
WU = UNITS // NW       # 1024 units per wave

BF16 = mybir.dt.bfloat16
FP32 = mybir.dt.float32
FP32R = mybir.dt.float32r

bf16 = ml_dtypes.bfloat16


def _build_nc(use_b, use_bias):
    nc = bacc.Bacc("TRN2", target_bir_lowering=False, debug=False, enable_asserts=False)

    wctx = nc.declare_dram_parameter("wctx", [P, KC_CTX * (RANK + BS)], BF16, isOutput=False)
    u8 = nc.declare_dram_parameter("u8", [NQ, P, KPQ * RANK], BF16, isOutput=False)
    xg = nc.declare_dram_parameter("xg", [NSB, NQ, P, KPQ * BSB], BF16, isOutput=False)
    vt4 = nc.declare_dram_parameter("vt4", [NW, P, RT * WU], BF16, isOutput=False)
    if use_b:
        bvec = nc.declare_dram_parameter("bvec", [P, RT], FP32, isOutput=False)
    if use_bias:
        brow = nc.declare_dram_parameter("brow", [1, P + UNITS], FP32R, isOutput=False)
    out_d = nc.declare_dram_parameter("out_d", [BS, UNITS], BF16, isOutput=True)

    with tile.TileContext(nc) as tc:
        with (
            tc.tile_pool(name="small", bufs=1) as small,
            tc.tile_pool(name="stream", bufs=1) as stream,
            tc.tile_pool(name="acts", bufs=1) as acts,
            tc.tile_pool(name="ostage", bufs=4) as ostage,
            tc.tile_pool(name="pchi", bufs=1, space="PSUM") as pchi,
            tc.tile_pool(name="pt", bufs=1, space="PSUM") as pt,
            tc.tile_pool(name="pout", bufs=2, space="PSUM") as pout,
        ):
            # ---- SBUF tiles ----
            wctx_sb = small.tile([P, KC_CTX * (RANK + BS)], BF16, tag="wctx")
            u_sb = small.tile([P, NQ, KPQ * RANK], BF16, tag="u")
            x_sb = [[stream.tile([P, KPQ * BSB], BF16, tag=f"x{s}{g}", name=f"x{s}{g}")
                     for g in range(NQ)] for s in range(NSB)]
            vt_sb = small.tile([P, NW, RT * WU], BF16, tag="vt")
            if use_b:
                bvec_sb = small.tile([P, RT], FP32, tag="bvec")
            if use_bias:
                brow_sb = small.tile([1, P + UNITS], FP32R, tag="brow")
            s_chi = acts.tile([P, RT, BS], FP32, tag="schi")
            t_sb = [acts.tile([P, RT, BSB], BF16, tag=f"tsb{s}", name=f"tsb{s}")
                    for s in range(NSB)]
            junk = acts.tile([P, P], BF16, tag="junk")

            # ---- DMA issue queues (per-ring order == consumption order).
            # u/xA piece-pairs alternate rings so mm1-A piece k needs only
            # the k-th completion on each ring; wctx mid-stream (chi is gap
            # filler); xB next so mm1-B continues the PE's stream-paced run;
            # VT LAST -- by then the PE has ~17us of mm2 work left, which
            # runs dense against VT's 5.5us arrival.
            nc.sync.dma_start(u_sb[:, 0, :], u8[0])
            nc.scalar.dma_start(x_sb[0][0][:], xg[0, 0])
            nc.sync.dma_start(x_sb[0][1][:], xg[0, 1])
            nc.scalar.dma_start(u_sb[:, 1, :], u8[1])
            nc.sync.dma_start(u_sb[:, 2, :], u8[2])
            nc.scalar.dma_start(x_sb[0][2][:], xg[0, 2])
            nc.sync.dma_start(x_sb[0][3][:], xg[0, 3])
            nc.scalar.dma_start(u_sb[:, 3, :], u8[3])
            if use_b:
                nc.sync.dma_start(bvec_sb[:], bvec[:])
            if use_bias:
                nc.sync.dma_start(brow_sb[:], brow[:])
            nc.scalar.dma_start(wctx_sb[:], wctx[:])
            nc.sync.dma_start(u_sb[:, 4, :], u8[4])
            nc.scalar.dma_start(x_sb[0][4][:], xg[0, 4])
            nc.sync.dma_start(x_sb[0][5][:], xg[0, 5])
            nc.scalar.dma_start(u_sb[:, 5, :], u8[5])
            nc.sync.dma_start(u_sb[:, 6, :], u8[6])
            nc.scalar.dma_start(x_sb[0][6][:], xg[0, 6])
            nc.sync.dma_start(x_sb[0][7][:], xg[0, 7])
            nc.scalar.dma_start(u_sb[:, 7, :], u8[7])
            for g in range(NQ):
                eng = nc.scalar if g % 2 == 0 else nc.sync
                eng.dma_start(x_sb[1][g][:], xg[1, g])
            nc.scalar.dma_start(vt_sb[:, 0, :], vt4[0])
            nc.sync.dma_start(vt_sb[:, 1, :], vt4[1])
            nc.scalar.dma_start(vt_sb[:, 2, :], vt4[2])
            nc.sync.dma_start(vt_sb[:, 3, :], vt4[3])

            # ---- PE warm-up: keep the HAM activity window busy from t=0
            # so the clock gate lifts to 2.4 GHz before the real stream.
            nc.gpsimd.memset(junk[:], 0.0)
            warm_ps = pchi.tile([P, BS], FP32, tag="chi", name="warm_ps")
            for _ in range(16):
                nc.tensor.matmul(
                    warm_ps[:, :P], junk[:], junk[:],
                    start=True, stop=True, skip_group_check=True,
                )

            # ---- chi' = sigmoid(W.T @ ctxT + B)  (S folded into U) ----
            for rt in range(RT):
                psum_chi = pchi.tile([P, BS], FP32, tag="chi", name=f"pchi{rt}")
                for k in range(KC_CTX):
                    base = k * (RANK + BS)
                    nc.tensor.matmul(
                        psum_chi[:],
                        wctx_sb[:, base + rt * P: base + (rt + 1) * P],
                        wctx_sb[:, base + RANK: base + RANK + BS],
                        start=(k == 0), stop=(k == KC_CTX - 1),
                        skip_group_check=True,
                    )
                nc.scalar.activation(
                    s_chi[:, rt, :], psum_chi[:],
                    mybir.ActivationFunctionType.Sigmoid,
                    bias=(bvec_sb[:, rt:rt + 1] if use_b else 0.0), scale=1.0,
                )

            psum_t = [pt.tile([P, RT * BSB], FP32, tag=f"pt{s}", name=f"pt{s}")
                      for s in range(NSB)]

            def emit_mm1_piece(s, q):
                # psum_t[s][:, rt*BSB:(rt+1)*BSB] += U'_k.T @ x_k, k in piece q.
                # Both rank-half groups share ONE psum bank; start=True clears
                # has_written BANK-wide, so only the very first matmul carries
                # it (the rt1 group's k=0 lands on cleared bits and start=False
                # already overwrites).
                for j in range(KPQ):
                    k = q * KPQ + j
                    for rt in range(RT):
                        nc.tensor.matmul(
                            psum_t[s][:, rt * BSB:(rt + 1) * BSB],
                            u_sb[:, q, j * RANK + rt * P: j * RANK + (rt + 1) * P],
                            x_sb[s][q][:, j * BSB:(j + 1) * BSB],
                            start=(k == 0 and rt == 0),
                            stop=(k == KC_IN - 1),
                            skip_group_check=True,
                        )

            def emit_tprime(s):
                for rt in range(RT):
                    nc.vector.tensor_mul(
                        t_sb[s][:, rt, :],
                        psum_t[s][:, rt * BSB:(rt + 1) * BSB],
                        s_chi[:, rt, s * BSB:(s + 1) * BSB],
                    )

            def emit_mm2_wave(s, w, bt, widx):
                # out[bt-rows, wave-units] = t'.T @ VT (+ 2*bias), relu, DMA.
                # One accumulation group per 2-bank tile (N=1024 moving).
                pw = pout.tile([P, WU], FP32, tag="po")
                for rt in range(RT):
                    for h in range(2):
                        nc.tensor.matmul(
                            pw[:, h * 512:(h + 1) * 512],
                            t_sb[s][:, rt, bt * P:(bt + 1) * P],
                            vt_sb[:, w, rt * WU + h * 512: rt * WU + (h + 1) * 512],
                            start=(rt == 0),
                            stop=(rt == RT - 1 and not use_bias),
                            skip_group_check=True,
                        )
                if use_bias:
                    for h in range(2):
                        nc.tensor.matmul(
                            pw[:, h * 512:(h + 1) * 512],
                            brow_sb[:, 0:P],
                            brow_sb[:, P + w * WU + h * 512: P + w * WU + (h + 1) * 512],
                            start=False, stop=True,
                            skip_group_check=True,
                        )
                o_sb = ostage.tile([P, WU], BF16, tag="osb")
                if widx % 2 == 0:
                    nc.scalar.activation(
                        o_sb[:], pw[:], mybir.ActivationFunctionType.Relu,
                    )
                else:
                    nc.vector.tensor_scalar(
                        o_sb[:], pw[:], 0.0, None, op0=mybir.AluOpType.max,
                    )
                rows = slice(s * BSB + bt * P, s * BSB + (bt + 1) * P)
                cols = slice(w * WU, (w + 1) * WU)
                eng = nc.scalar if (s == 0) != (widx % 2 == 0) else nc.sync
                eng.dma_start(out_d[rows, cols], o_sb[:])

            # mm1-A then mm1-B ride the DMA-paced stream (chi fills gaps);
            # all of mm2 then runs dense, paced only by VT's arrival.
            for q in range(NQ):
                emit_mm1_piece(0, q)
            emit_tprime(0)
            for q in range(NQ):
                emit_mm1_piece(1, q)
            emit_tprime(1)
            widx = 0
            for w in range(NW):
                for s in range(NSB):
                    for bt in range(NBT):
                        emit_mm2_wave(s, w, bt, widx)
                        widx += 1

    nc.finalize()
    return nc


_NC_CACHE = {}


def _get_nc(use_b=False, use_bias=False):
    key = (use_b, use_bias)
    if key not in _NC_CACHE:
        _NC_CACHE[key] = _build_nc(use_b, use_bias)
    return _NC_CACHE[key]


def _round_fp32r(a):
    """Round fp32 to the fp32r grid (11-bit mantissa; low 12 bits zero)."""
    u = np.ascontiguousarray(a, dtype=np.float32).view(np.uint32)
    r = (u + np.uint32(0x7FF) + ((u >> np.uint32(12)) & np.uint32(1))) & np.uint32(0xFFFFF000)
    return r.view(np.float32)


def build(inputs, context, U, S, V, W, B, bias):
    """Host-side packing: returns (nc, in_maps)."""
    use_b = bool(np.any(np.asarray(B)))
    use_bias = bool(np.any(np.asarray(bias)))

    # U with S folded into its columns, chunked for the stream:
    # u8[q, p, j*RANK + r] = (U*S)[(q*KPQ+j)*128 + p, r]
    US = (np.asarray(U, np.float32) * np.asarray(S, np.float32)[None, :]).astype(bf16)
    u8 = np.ascontiguousarray(
        US.reshape(NQ, KPQ, P, RANK).transpose(0, 2, 1, 3).reshape(NQ, P, KPQ * RANK)
    )

    # VT pieces: vt4[c, p, rt*WU + m'] = V.T[rt*128 + p, c*WU + m']
    VTb = np.asarray(V, np.float32).T.astype(bf16)          # [RANK, UNITS]
    vt4 = np.ascontiguousarray(
        VTb.reshape(RT, P, NW, WU).transpose(2, 1, 0, 3).reshape(NW, P, RT * WU)
    )

    Wk = np.asarray(W, np.float32).astype(bf16).reshape(KC_CTX, P, RANK)
    ctxT = np.asarray(context, np.float32).astype(bf16).T   # [N_CTX, B_SZ]
    xT = np.asarray(inputs, np.float32).astype(bf16).T      # [N_IN, B_SZ]

    bvec = np.ascontiguousarray(np.asarray(B, np.float32).reshape(RT, P).T)
    brow = np.empty((1, P + UNITS), np.float32)
    brow[0, :P] = 1.0
    brow[0, P:] = 2.0 * np.asarray(bias, np.float32)
    brow = _round_fp32r(brow)

    in_maps = []
    for c in range(N_CORES):
        sl = slice(c * BS, (c + 1) * BS)
        # wctx[p, k*(RANK+BS) + ...] = [W_k | ctx_k] per contraction chunk
        wc = np.empty((KC_CTX, P, RANK + BS), bf16)
        wc[:, :, :RANK] = Wk
        wc[:, :, RANK:] = ctxT[:, sl].reshape(KC_CTX, P, BS)
        wctx = np.ascontiguousarray(
            wc.transpose(1, 0, 2).reshape(P, KC_CTX * (RANK + BS))
        )
        # xg[s, g, p, j*BSB + b'] = xT[(g*KPQ+j)*128 + p, c*BS + s*BSB + b']
        xc = xT[:, sl]                                       # [N_IN, BS]
        xgc = np.ascontiguousarray(
            xc.reshape(NQ, KPQ, P, NSB, BSB)
              .transpose(3, 0, 2, 1, 4)
              .reshape(NSB, NQ, P, KPQ * BSB)
        )
        m = {"wctx": wctx, "u8": u8, "xg": xgc, "vt4": vt4}
        if use_b:
            m["bvec"] = bvec
        if use_bias:
            m["brow"] = brow
        in_maps.append(m)
    return _get_nc(use_b, use_bias), in_maps


def gather_out(results):
    out = np.empty((B_SZ, UNITS), dtype=np.float32)
    for c in range(N_CORES):
        out[c * BS:(c + 1) * BS, :] = results[c]["out_d"].astype(np.float32)
    return out


def kernel(inputs, context, U, S, V, W, B, bias):
    nc, in_maps = build(inputs, context, U, S, V, W, B, bias)
    res = run_bass_kernel_spmd(nc, in_maps, list(range(N_CORES)))
    return gather_out(res.results)
